# revision 5
# baseline (speedup 1.0000x reference)
"""Trainium2 Bass kernel for nn_CachedMoEExperts (MoE routing, E=16, top-4).

Strategy (expert-parallel, host-side dispatch):
  - Host computes the (tiny) router: softmax -> top-4 -> renormalize.
  - Tokens are gathered per expert on the host; experts are paired
    big-with-small and assigned 2 per NeuronCore (16 experts / 8 cores).
  - Each core runs the expert FFN y = gate * (w2 @ silu(w1 @ x_g^T)) for its
    two experts in fp16 (full-rate PE + fast weight load) on zero-padded
    token batches (slot capacities C0/C1, fixed at compile time).
  - Weights and gathered activations are host-packed into the exact SBUF
    tile layouts so every DMA is one fully-contiguous transfer.
  - Host scatter-adds the per-expert outputs back into the [T, H] result.

mm1 is column-chunk-major with the slot's w1 tiles resident in SBUF: the
first accumulation chain needs only w1-m0 plus one kt-block of chunk 0
(~0.8 MB) instead of racing the full 4.6 MB xg delivery, which removes the
~15 us DMA lead-in the kt-major order had.  All large loads ride the two
HWDGE rings (scalar for slot0 xg + y stores, sync for weights + slot1 xg,
ordered so slot1 traffic queues behind the slot0-critical bytes); only the
tiny gate vectors use the slow SWDGE (gpsimd) path.
"""

from contextlib import ExitStack

import numpy as np

import concourse.bacc as bacc
import concourse.bass as bass
import concourse.mybir as mybir
import concourse.tile as tile
from concourse.bass_utils import run_bass_kernel_spmd

F32 = mybir.dt.float32
FP16 = mybir.dt.float16

NUM_EXPERTS = 16
TOP_K = 4
HIDDEN = 2048
INTER = 1408
TOKENS = 4096
N_CORES = 8

KT1 = HIDDEN // 128  # 16 contraction tiles for mm1
MT1 = INTER // 128   # 11 output-row tiles for mm1
KT2 = INTER // 128   # 11 contraction tiles for mm2
MT2 = HIDDEN // 128  # 16 output-row tiles for mm2

# Default slot capacities (tokens routed per expert; avg load is T*K/E=1024).
CAP0_DEFAULT = 1120  # the 8 most-loaded experts
CAP1_DEFAULT = 1024  # the 8 least-loaded experts

_PROGRAM_CACHE: dict = {}


def _ceil32(n: int) -> int:
    return max(128, (int(n) + 31) // 32 * 32)


def _plan_chunks(C: int):
    """Split the token capacity into moving-dim chunks of <=512 (PSUM bank
    limit for fp32 accumulation)."""
    chunks = []
    off, rem = 0, C
    while rem > 0:
        sz = min(512, rem)
        chunks.append((off, sz))
        off += sz
        rem -= sz
    return chunks


def _build_program(C0: int, C1: int, reps: int = 1):
    caps = (C0, C1)
    nc = bacc.Bacc("TRN2", debug=False, target_bir_lowering=False)

    xg_d = {}
    for s in range(2):
        for ci, (off, w) in enumerate(_plan_chunks(caps[s])):
            xg_d[(s, ci)] = nc.dram_tensor(
                f"xg{s}c{ci}", (128, KT1, w), FP16, kind="ExternalInput"
            )
    g_d = [
        nc.dram_tensor(f"g{s}", (128, caps[s]), F32, kind="ExternalInput")
        for s in range(2)
    ]
    y_d = [
        nc.dram_tensor(f"y{s}", (HIDDEN, caps[s]), FP16, kind="ExternalOutput")
        for s in range(2)
    ]
    w1_d = nc.dram_tensor(
        "w1p", (2, MT1, 128, KT1, 128), FP16, kind="ExternalInput"
    )
    w2_d = nc.dram_tensor(
        "w2p", (2, MT2, 128, KT2, 128), FP16, kind="ExternalInput"
    )

    with tile.TileContext(nc) as tc, ExitStack() as ctx:
        xgp = ctx.enter_context(tc.tile_pool(name="xg", bufs=1))
        wp = ctx.enter_context(tc.tile_pool(name="w", bufs=1))
        h1p = ctx.enter_context(tc.tile_pool(name="h1", bufs=1))
        gp = ctx.enter_context(tc.tile_pool(name="g", bufs=1))
        pp = ctx.enter_context(
            tc.tile_pool(name="psum", bufs=2, space=bass.MemorySpace.PSUM)
        )
        op = ctx.enter_context(tc.tile_pool(name="out", bufs=3))
        if reps > 1:
            ctx.enter_context(tc.For_i(0, reps, 1))

        for s in range(2):
            C = caps[s]
            chunks = _plan_chunks(C)

            # xg chunk loads.  slot0 rides the scalar/Act HWDGE ring with
            # fine kt sub-blocks on chunk 0 so the first chain starts ~2us
            # in; slot1 rides the sync ring *behind* slot0's weights (it is
            # not needed until mm1-s1, ~250us in).
            xg_tiles = []
            for ci, (off, w) in enumerate(chunks):
                t = xgp.tile(
                    [128, KT1, w], FP16, tag=f"xg{s}c{ci}", name=f"xg_s{s}c{ci}"
                )
                if s == 0 and ci == 0:
                    blocks = [(0, 1), (1, 1), (2, 2), (4, 4), (8, 8)]
                elif s == 0:
                    # finer blocks so w1 tiles interleave on the DMA fabric
                    blocks = [(0, 8), (8, 8)]
                else:
                    blocks = [(0, KT1)]
                eng = nc.scalar if s == 0 else nc.sync
                for kq, kn in blocks:
                    eng.dma_start(
                        t[:, kq : kq + kn, :], xg_d[(s, ci)].ap()[:, kq : kq + kn, :]
                    )
                xg_tiles.append(t)
            g_t = gp.tile([128, C], F32, tag=f"g{s}", name=f"g_s{s}")
            nc.gpsimd.dma_start(g_t[:], g_d[s].ap()[:, :])

            h1_tiles = [
                h1p.tile([128, C], FP16, tag=f"h1_{m}", name=f"h1_s{s}_{m}")
                for m in range(MT1)
            ]

            # w1 tiles for this slot stay resident across all column chunks.
            w1_tiles = []
            for m in range(MT1):
                wt = wp.tile(
                    [128, KT1, 128], FP16, tag=f"w1m{m}", bufs=1,
                    name=f"w1_s{s}_{m}",
                )
                nc.sync.dma_start(wt[:], w1_d.ap()[s, m])
                w1_tiles.append(wt)

            # mm1 + silu: h1[i, t] = silu(sum_h w1[i, h] * x[t, h])
            # column-chunk-major: chunk ci needs only xg chunk ci, so the
            # PE starts as soon as w1-m0 and the first kt block land.
            for ci, (off, w) in enumerate(chunks):
                for m0 in range(0, MT1, 2):
                    ms = [m for m in (m0, m0 + 1) if m < MT1]
                    pss = {
                        m: pp.tile([128, 512], F32, tag=f"p{mi}", bufs=2,
                                   name=f"ps1_s{s}_{ci}_{m}")
                        for mi, m in enumerate(ms)
                    }
                    for kt in range(KT1):
                        for m in ms:
                            nc.tensor.matmul(
                                pss[m][:, :w],
                                w1_tiles[m][:, kt, :],
                                xg_tiles[ci][:, kt, :],
                                start=(kt == 0),
                                stop=(kt == KT1 - 1),
                            )
                    for m in ms:
                        nc.scalar.activation(
                            h1_tiles[m][:, off : off + w],
                            pss[m][:, :w],
                            mybir.ActivationFunctionType.Silu,
                        )

            # mm2 + gate: y[hh, t] = g[t] * sum_i w2[hh, i] * h1[i, t]
            # psum tags p0/p1 are shared with mm1 (never live at the same
            # time); the third chunk gets its own tag q2.
            for m2 in range(MT2):
                wt2 = wp.tile([128, KT2, 128], FP16, tag="w2", bufs=4,
                              name=f"w2_s{s}_{m2}")
                nc.sync.dma_start(wt2[:], w2_d.ap()[s, m2])
                tags = ["p0", "p1", "q2"]
                pss = [
                    pp.tile([128, 512], F32, tag=tags[ci], bufs=2,
                            name=f"ps2_s{s}_{m2}_{ci}")
                    for ci, (off, szn) in enumerate(chunks)
                ]
                for kt in range(KT2):
                    for ci, (off, szn) in enumerate(chunks):
                        nc.tensor.matmul(
                            pss[ci][:, :szn],
                            wt2[:, kt, :],
                            h1_tiles[kt][:, off : off + szn],
                            start=(kt == 0),
                            stop=(kt == KT2 - 1),
                        )
                ot = op.tile([128, C], FP16, tag="out", name=f"ot_s{s}_{m2}")
                for ci, (off, szn) in enumerate(chunks):
                    nc.vector.tensor_mul(
                        ot[:, off : off + szn], pss[ci][:, :szn],
                        g_t[:, off : off + szn],
                    )
                    # per-chunk store: chunk ci's DMA overlaps chunk ci+1's
                    # gate-multiply, shortening the end-of-execution tail
                    nc.scalar.dma_start(
                        y_d[s].ap()[m2 * 128 : (m2 + 1) * 128, off : off + szn],
                        ot[:, off : off + szn],
                    )

    nc.compile()
    return nc


def _get_program(C0: int, C1: int):
    key = (C0, C1)
    if key not in _PROGRAM_CACHE:
        _PROGRAM_CACHE[key] = _build_program(C0, C1)
    return _PROGRAM_CACHE[key]


def build_reps_program(in_maps, reps: int):
    """Benchmark hook: rebuild the same program with the body wrapped in a
    hardware For_i(0, reps) loop.  Capacities are recovered from the staged
    input shapes."""
    C0 = sum(v.shape[-1] for k, v in in_maps[0].items() if k.startswith("xg0"))
    C1 = sum(v.shape[-1] for k, v in in_maps[0].items() if k.startswith("xg1"))
    return _build_program(C0, C1, reps=reps)


def _route(router_logits: np.ndarray):
    """softmax -> top-4 (desc, ties by lower index) -> renormalize; matches
    jax.nn.softmax + jax.lax.top_k semantics in float32."""
    logits = router_logits.astype(np.float32, copy=False)
    m = logits.max(axis=-1, keepdims=True)
    e = np.exp(logits - m)
    probs = e / e.sum(axis=-1, keepdims=True)
    top_idx = np.argsort(-probs, axis=-1, kind="stable")[:, :TOP_K]
    top_vals = np.take_along_axis(probs, top_idx, axis=-1)
    top_vals = top_vals / top_vals.sum(axis=-1, keepdims=True)
    return top_idx.astype(np.int64), top_vals.astype(np.float32)


def _pack_w1(w1e: np.ndarray) -> np.ndarray:
    # [I, H] -> [MT1, 128, KT1, 128] with [m, p, kt, j] = w1e[m*128+j, kt*128+p]
    return np.ascontiguousarray(
        w1e.reshape(MT1, 128, KT1, 128).transpose(0, 3, 2, 1).astype(np.float16)
    )


def _pack_w2(w2e: np.ndarray) -> np.ndarray:
    # [H, I] -> [MT2, 128, KT2, 128] with [m, p, kt, j] = w2e[m*128+j, kt*128+p]
    return np.ascontiguousarray(
        w2e.reshape(MT2, 128, KT2, 128).transpose(0, 3, 2, 1).astype(np.float16)
    )


def _pack_xg_chunks(xsel: np.ndarray, C: int):
    # xsel [n, H] -> per-chunk [128, KT1, w] with [p, kt, t] = xsel[t, kt*128+p]
    n = xsel.shape[0]
    full = np.zeros((128, KT1, C), np.float16)
    full[:, :, :n] = xsel.T.reshape(KT1, 128, n).transpose(1, 0, 2)
    return [
        np.ascontiguousarray(full[:, :, off : off + w])
        for off, w in _plan_chunks(C)
    ]


def _prepare(x, router_logits, w1, w2):
    x = np.ascontiguousarray(np.asarray(x, dtype=np.float32))
    router_logits = np.asarray(router_logits, dtype=np.float32)
    w1 = np.asarray(w1, dtype=np.float32)
    w2 = np.asarray(w2, dtype=np.float32)
    T = x.shape[0]

    top_idx, top_gates = _route(router_logits)

    flat_e = top_idx.ravel()
    flat_t = np.repeat(np.arange(T), TOP_K)
    flat_g = top_gates.ravel()
    order = np.argsort(flat_e, kind="stable")
    st, sg = flat_t[order], flat_g[order]
    counts = np.bincount(flat_e, minlength=NUM_EXPERTS)
    starts = np.concatenate([[0], np.cumsum(counts)])
    toks = [st[starts[e] : starts[e + 1]] for e in range(NUM_EXPERTS)]
    gs = [sg[starts[e] : starts[e + 1]] for e in range(NUM_EXPERTS)]

    # pair the most-loaded expert with the least-loaded, 2 experts per core
    rank = np.argsort(-counts, kind="stable")
    big = rank[:N_CORES]
    small = rank[N_CORES:][::-1]  # big[i] pairs with small[i]

    C0 = max(CAP0_DEFAULT, _ceil32(counts[big].max()))
    C1 = max(CAP1_DEFAULT, _ceil32(counts[small].max()))
    nc = _get_program(C0, C1)

    in_maps = []
    for c in range(N_CORES):
        pair = (int(big[c]), int(small[c]))
        caps = (C0, C1)
        im = {}
        for s, e in enumerate(pair):
            n = int(counts[e])
            for ci, xc in enumerate(_pack_xg_chunks(x[toks[e]], caps[s])):
                im[f"xg{s}c{ci}"] = xc
            g = np.zeros((caps[s],), np.float32)
            g[:n] = gs[e]
            im[f"g{s}"] = np.broadcast_to(g, (128, caps[s])).copy()
        im["w1p"] = np.stack([_pack_w1(w1[e]) for e in pair])
        im["w2p"] = np.stack([_pack_w2(w2[e]) for e in pair])
        in_maps.append(im)

    meta = dict(T=T, counts=counts, toks=toks, big=big, small=small)
    return nc, in_maps, meta


def _combine(results, meta):
    out = np.zeros((meta["T"], HIDDEN), np.float32)
    for c in range(N_CORES):
        for s, e in enumerate((int(meta["big"][c]), int(meta["small"][c]))):
            n = int(meta["counts"][e])
            y = results[c][f"y{s}"]  # [HIDDEN, Cs], already gate-scaled
            out[meta["toks"][e]] += y[:, :n].T.astype(np.float32)
    return out


def kernel(x, router_logits, w1, w2):
    nc, in_maps, meta = _prepare(x, router_logits, w1, w2)
    res = run_bass_kernel_spmd(nc, in_maps, core_ids=list(range(N_CORES)))
    kernel._last_results = res
    return _combine(res.results, meta)
